# revision 1
# baseline (speedup 1.0000x reference)
"""Trainium2 Bass kernel for nn_ConvCapsuleLayer3D.

Self-contained: takes FULL inputs x[32,32,32,8,16], W[16,3,3,1,256], b[16,16,1,1],
returns FULL output [32,30,30,16,16] (fp32). Data-parallel over batch across 8
NeuronCores (4 samples each).

Per-sample plan (all fp32):
  conv:   im2col [144,(d,hw)=7200] built by 72 DMAs straight from HBM; matmul
          with im2col slices as stationary operand -> votes [hw_chunk, (i,o,a)]
          in PSUM, drained to SBUF by ScalarE.
  routing (3 iters), layout = hw on partitions (h-aligned chunks of 120/60):
          softmax over o, preact = sum_i r*V (DVE mul + DVE reduce), bias add,
          squash over w via tiny mask matmuls on TensorE (sum over the 30
          w-positions that live inside each chunk's partitions), act = preact *
          scale, agreement logits update (DVE mul + reduce over a).
"""
import os
import sys

import numpy as np

sys.path.insert(0, "/opt/trn_rl_repo")

# --- problem constants (hardcoded; kernel.py must not read /root/problem) ---
B, H, WD, IC, IA = 32, 32, 32, 8, 16
OC, NA = 16, 16
K = 3
HC, WC = H - K + 1, WD - K + 1       # 30, 30
HW = HC * WC                         # 900
CO = OC * NA                         # 256
NCORES = 8
NSAMP = B // NCORES                  # 4
EPS = 1e-7
ROUTINGS = 3

CP_FULL = 120                        # 4 h-rows per chunk
CHUNKS = [(c, CP_FULL, 4) for c in range(7)] + [(7, 60, 2)]  # (c, cp, nj)


def _build_body(ctx, tc, x_ap, w_ap, b_ap, out_ap):
    import concourse.bass as bass
    import concourse.mybir as mybir

    nc = tc.nc
    f32 = mybir.dt.float32
    Alu = mybir.AluOpType
    Act = mybir.ActivationFunctionType
    X = mybir.AxisListType.X

    def pap(t, part, dims, off=0):
        """AP over tile t: partitions [0,part), free dims [[step,count],...] (elements)."""
        a = t if isinstance(t, bass.AP) else t.ap()
        pstep = a.ap[0][0]  # partition pitch in elements (may be padded)
        return bass.AP(tensor=a.tensor, offset=a.offset + off,
                       ap=[[pstep, part]] + dims)

    consts = ctx.enter_context(tc.tile_pool(name="consts", bufs=1))
    imc_pool = ctx.enter_context(tc.tile_pool(name="imc", bufs=2))
    v_pool = ctx.enter_context(tc.tile_pool(name="votes", bufs=3))
    vr_pool = ctx.enter_context(tc.tile_pool(name="vr", bufs=3))
    small = ctx.enter_context(tc.tile_pool(name="small", bufs=4))
    acts = ctx.enter_context(tc.tile_pool(name="acts", bufs=4))
    psum_c = ctx.enter_context(tc.tile_pool(name="psc", bufs=4, space="PSUM"))
    psum_s = ctx.enter_context(tc.tile_pool(name="pss", bufs=2, space="PSUM"))
    psum_b = ctx.enter_context(tc.tile_pool(name="psb", bufs=2, space="PSUM"))

    # ---- constants ----
    wa = consts.tile([128, CO], f32, tag="wa")      # K rows (kh,kw,kd) 0..127
    wb = consts.tile([16, CO], f32, tag="wb")       # K rows 128..143
    for kh in range(K):
        for kw in range(K):
            blk = kh * K + kw
            src = bass.AP(tensor=w_ap.tensor, offset=w_ap.offset + kh * 768 + kw * 256,
                          ap=[[2304, 16], [1, 256]])
            if blk < 8:
                nc.sync.dma_start(out=wa[blk * 16:(blk + 1) * 16, :], in_=src)
            else:
                nc.sync.dma_start(out=wb[:, :], in_=src)

    bfull = consts.tile([128, CO], f32, tag="bfull")
    nc.sync.dma_start(out=bfull[:, :],
                      in_=bass.AP(tensor=b_ap.tensor, offset=b_ap.offset,
                                  ap=[[0, 128], [1, 256]]))

    zero_t = consts.tile([128, 1], f32, tag="zero")
    nc.vector.memset(zero_t[:, :], 0.0)
    eps_t = consts.tile([128, 1], f32, tag="eps")
    nc.vector.memset(eps_t[:, :], EPS)

    # mask[p,j] = (p//30 == j); sel[j,p] = (p//30 == j). Engine writes must
    # start at partition 0 (32-aligned), so build via iota(p-30j) + compares.
    i32 = mybir.dt.int32
    mask = consts.tile([CP_FULL, 4], f32, tag="mask")
    sel = consts.tile([4, CP_FULL], f32, tag="sel")
    mi = consts.tile([CP_FULL, 4], i32, tag="mi")
    si = consts.tile([4, CP_FULL], i32, tag="si")
    mf = consts.tile([CP_FULL, 4], f32, tag="mf")
    sf = consts.tile([4, CP_FULL], f32, tag="sf")
    nc.gpsimd.iota(mi[:, :], pattern=[[-30, 4]], base=0, channel_multiplier=1)
    nc.gpsimd.iota(si[:, :], pattern=[[1, CP_FULL]], base=0, channel_multiplier=-30)
    nc.vector.tensor_copy(out=mf[:, :], in_=mi[:, :])
    nc.vector.tensor_copy(out=sf[:, :], in_=si[:, :])
    for dst, src in ((mask, mf), (sel, sf)):
        ge = consts.tile(list(dst.shape), f32, tag=f"ge{dst.shape[0]}")
        lt = consts.tile(list(dst.shape), f32, tag=f"lt{dst.shape[0]}")
        nc.vector.tensor_scalar(out=ge[:, :], in0=src[:, :], scalar1=0.0,
                                scalar2=None, op0=Alu.is_ge)
        nc.vector.tensor_scalar(out=lt[:, :], in0=src[:, :], scalar1=30.0,
                                scalar2=None, op0=Alu.is_lt)
        nc.vector.tensor_tensor(out=dst[:, :], in0=ge[:, :], in1=lt[:, :],
                                op=Alu.mult)

    for s in range(NSAMP):
        # ---- im2col DMAs: imA [128, (d,h',w')], imB [16, (d,h',w')] ----
        imA = imc_pool.tile([128, IC, HC, WC], f32, tag="imA")
        imB = imc_pool.tile([16, IC, HC, WC], f32, tag="imB")
        xoff = x_ap.offset + s * (H * WD * IC * IA)
        for kh in range(K):
            for kw in range(K):
                blk = kh * K + kw
                for d in range(IC):
                    src = bass.AP(tensor=x_ap.tensor,
                                  offset=xoff + d * (IA * H * WD) + kh * WD + kw,
                                  ap=[[H * WD, IA], [WD, HC], [1, WC]])
                    if blk < 8:
                        nc.sync.dma_start(out=imA[blk * 16:(blk + 1) * 16, d, :, :], in_=src)
                    else:
                        nc.sync.dma_start(out=imB[:, d, :, :], in_=src)

        for (c, cp, nj) in CHUNKS:
            # ---- conv for this chunk: votes V [cp, (i,o,a)] ----
            V = v_pool.tile([CP_FULL, IC, OC, NA], f32, tag="V")
            for d in range(IC):
                pc = psum_c.tile([CP_FULL, CO], f32, tag="pc")
                nc.tensor.matmul(pc[:cp, :], imA[:, d, 4 * c:4 * c + nj, :],
                                 wa[:, :], start=True, stop=False)
                nc.tensor.matmul(pc[:cp, :], imB[:, d, 4 * c:4 * c + nj, :],
                                 wb[:, :], start=False, stop=True)
                nc.scalar.copy(out=V[:cp, d, :, :], in_=pc[:cp, :])

            # ---- routing ----
            L = small.tile([CP_FULL, IC, OC], f32, tag="L")
            nc.gpsimd.memset(L[:cp, :, :], 0.0)
            for it in range(ROUTINGS):
                # softmax over o (free)
                e = small.tile([CP_FULL, IC, OC], f32, tag="e")
                nc.scalar.activation(out=e[:cp, :, :], in_=L[:cp, :, :], func=Act.Exp,
                                     bias=zero_t[:cp, :])
                ssum = small.tile([CP_FULL, IC], f32, tag="ssum")
                nc.vector.tensor_reduce(out=ssum[:cp, :], in_=e[:cp, :, :],
                                        axis=X, op=Alu.add)
                srec = small.tile([CP_FULL, IC], f32, tag="srec")
                nc.vector.reciprocal(out=srec[:cp, :], in_=ssum[:cp, :])
                r = small.tile([CP_FULL, IC, OC], f32, tag="r")
                # iterate (o, i): innermost steps nonzero on all operands
                nc.vector.tensor_tensor(
                    out=pap(r, cp, [[1, OC], [OC, IC]]),
                    in0=pap(e, cp, [[1, OC], [OC, IC]]),
                    in1=pap(srec, cp, [[0, OC], [1, IC]]),
                    op=Alu.mult)
                # vr = V * r (broadcast over a); iterate (a, i, o)
                vr = vr_pool.tile([CP_FULL, IC, OC, NA], f32, tag="vr")
                nc.vector.tensor_tensor(
                    out=pap(vr, cp, [[1, NA], [CO, IC], [NA, OC]]),
                    in0=pap(V, cp, [[1, NA], [CO, IC], [NA, OC]]),
                    in1=pap(r, cp, [[0, NA], [OC, IC], [1, OC]]),
                    op=Alu.mult)
                # preact = sum_i vr  [cp, (o,a)]; reduce innermost=i
                preact = acts.tile([CP_FULL, CO], f32, tag="preact")
                nc.vector.tensor_reduce(
                    out=preact[:cp, :],
                    in_=pap(vr, cp, [[NA, OC], [1, NA], [CO, IC]]),
                    axis=X, op=Alu.add)
                nc.vector.tensor_tensor(out=preact[:cp, :], in0=preact[:cp, :],
                                        in1=bfull[:cp, :], op=Alu.add)
                # squash over w
                sq = acts.tile([CP_FULL, CO], f32, tag="sq")
                nc.scalar.activation(out=sq[:cp, :], in_=preact[:cp, :],
                                     func=Act.Square, bias=zero_t[:cp, :])
                s2 = psum_s.tile([4, CO], f32, tag="s2")
                nc.tensor.matmul(s2[:nj, :], mask[:cp, :nj], sq[:cp, :],
                                 start=True, stop=True)
                sqrt1 = small.tile([4, CO], f32, tag="sqrt1")
                nc.scalar.activation(out=sqrt1[:nj, :], in_=s2[:nj, :],
                                     func=Act.Sqrt, bias=eps_t[:nj, :])
                den = small.tile([4, CO], f32, tag="den")
                nc.vector.scalar_tensor_tensor(out=den[:nj, :], in0=s2[:nj, :],
                                               scalar=1.0, in1=sqrt1[:nj, :],
                                               op0=Alu.add, op1=Alu.mult)
                rden = small.tile([4, CO], f32, tag="rden")
                nc.vector.reciprocal(out=rden[:nj, :], in_=den[:nj, :])
                scl = small.tile([4, CO], f32, tag="scl")
                nc.vector.tensor_tensor(out=scl[:nj, :], in0=s2[:nj, :],
                                        in1=rden[:nj, :], op=Alu.mult)
                sclb = psum_b.tile([CP_FULL, CO], f32, tag="sclb")
                nc.tensor.matmul(sclb[:cp, :], sel[:nj, :cp], scl[:nj, :],
                                 start=True, stop=True)
                act = acts.tile([CP_FULL, CO], f32, tag="act")
                nc.vector.tensor_tensor(out=act[:cp, :], in0=preact[:cp, :],
                                        in1=sclb[:cp, :], op=Alu.mult)
                if it < ROUTINGS - 1:
                    # va = V * act (broadcast over i); natural (i,o,a) order
                    va = vr_pool.tile([CP_FULL, IC, OC, NA], f32, tag="vr")
                    nc.gpsimd.tensor_tensor(
                        out=va[:cp, :, :, :],
                        in0=V[:cp, :, :, :],
                        in1=pap(act, cp, [[0, IC], [NA, OC], [1, NA]]),
                        op=Alu.mult)
                    ld = small.tile([CP_FULL, IC, OC], f32, tag="ld")
                    nc.vector.tensor_reduce(out=ld[:cp, :, :], in_=va[:cp, :, :, :],
                                            axis=X, op=Alu.add)
                    nc.gpsimd.tensor_tensor(out=L[:cp, :, :], in0=L[:cp, :, :],
                                            in1=ld[:cp, :, :], op=Alu.add)
                else:
                    dst = bass.AP(tensor=out_ap.tensor,
                                  offset=out_ap.offset + s * (HW * CO) + c * (CP_FULL * CO),
                                  ap=[[CO, cp], [1, CO]])
                    nc.sync.dma_start(out=dst, in_=act[:cp, :])


_CACHED = None


def _build():
    global _CACHED
    if _CACHED is not None:
        return _CACHED
    from contextlib import ExitStack
    import concourse.bacc as bacc
    import concourse.mybir as mybir
    import concourse.tile as tile

    nc = bacc.Bacc("TRN2", target_bir_lowering=False, debug=False,
                   num_devices=NCORES)
    f32 = mybir.dt.float32
    x_t = nc.dram_tensor("x", [NSAMP, H, WD, IC, IA], f32, kind="ExternalInput")
    w_t = nc.dram_tensor("W", [IA, K, K, 1, CO], f32, kind="ExternalInput")
    b_t = nc.dram_tensor("b", [OC, NA, 1, 1], f32, kind="ExternalInput")
    out_t = nc.dram_tensor("out", [NSAMP, HC, WC, OC, NA], f32, kind="ExternalOutput")

    with tile.TileContext(nc) as tc:
        with ExitStack() as ctx:
            _build_body(ctx, tc, x_t.ap(), w_t.ap(), b_t.ap(), out_t.ap())
    nc.compile()
    _CACHED = nc
    return nc


def run(x, W, b, trace=False):
    from concourse.bass_utils import run_bass_kernel_spmd

    nc = _build()
    x = np.ascontiguousarray(x, np.float32)
    W = np.ascontiguousarray(W, np.float32)
    b = np.ascontiguousarray(b, np.float32)
    in_maps = [{"x": x[k * NSAMP:(k + 1) * NSAMP], "W": W, "b": b}
               for k in range(NCORES)]
    res = run_bass_kernel_spmd(nc, in_maps, core_ids=list(range(NCORES)),
                               trace=trace)
    out = np.concatenate([res.results[k]["out"] for k in range(NCORES)], axis=0)
    return out, res


def kernel(x, W, b):
    out, _ = run(x, W, b, trace=False)
    return out.astype(np.float32)


if __name__ == "__main__":
    nc = _build()
    print("built ok")



# revision 24
# speedup vs baseline: 1.6936x; 1.6936x over previous
"""Trainium2 Bass kernel for nn_ConvCapsuleLayer3D.

Self-contained: takes FULL inputs x[32,32,32,8,16], W[16,3,3,1,256], b[16,16,1,1],
returns FULL output [32,30,30,16,16] (fp32). Data-parallel over batch across 8
NeuronCores (4 samples each).

Host prep per core: stripped im2col replicas XA[(kh,kw<2,a)=96, (d,h',w')=7200]
and XB[(kh,kw=2,a)=48, 7200] so conv chunk windows are contiguous stationary
APs; weights pre-permuted to (a_out-major, o-minor) columns; fp16 constants.

Device per sample: conv = 2 accumulating f32r matmuls per (d, chunk) -> votes
V[hw_chunk, (i,a,o)] drained to fp16. Dynamic routing (3 iters, iteration-major
with iter-0 uniform-softmax shortcut) in fp16, with all 8 chunks of a sample
batched along the free dim so each step is one wide instruction: DVE 2x
tensor_tensor for heavy muls + tree-adds for the i/a reductions, squash row-sums
via 0/1 broadcast matmuls on TensorE, remaining elementwise spread over
GPSIMD/ACT. Chunks are 4 h-rows (120 positions); chunk 7 overlaps chunk 6 so
every chunk is full-width (rows 28-29 written from its lower half).
"""
import os
import sys

import numpy as np

sys.path.insert(0, "/opt/trn_rl_repo")

# --- problem constants (hardcoded; kernel.py must not read /root/problem) ---
B, H, WD, IC, IA = 32, 32, 32, 8, 16
OC, NA = 16, 16
K = 3
HC, WC = H - K + 1, WD - K + 1       # 30, 30
HW = HC * WC                         # 900
CO = OC * NA                         # 256
NCORES = 8
NSAMP = B // NCORES                  # 4
EPS = 1e-7
ROUTINGS = 3

CP = 120                             # 4 h-rows per chunk
NCH = 8
# chunk -> start position (chunk 7 overlaps 6; rows 28-29 come from its tail)
CSTART = [c * CP for c in range(7)] + [HW - CP]
VF = IC * CO                         # 2048 free elems per chunk of votes
SAMP_ELEMS = H * WD * IC * IA        # 131072
DCOL = 1024                          # per-(d,a) column run in x

INPUT_NAMES = ["XA", "XB", "WA", "WB", "B2", "BC"]


def make_in_maps(x, W, b):
    """Host prep: per-core input dicts for run_bass_kernel_spmd."""
    x = np.ascontiguousarray(x, np.float32)
    W = np.ascontiguousarray(W, np.float32)
    b = np.ascontiguousarray(b, np.float32)

    # weights: rows (kh, kw, a); cols permuted to co' = a_out*16 + o
    W5 = W[:, :, :, 0, :].reshape(IA, K, K, OC, NA)       # [a,kh,kw,o,ao]
    Wp = W5.transpose(1, 2, 0, 4, 3)                      # [kh,kw,a,ao,o]
    WA = np.ascontiguousarray(Wp[:, :2].reshape(96, CO))
    WB = np.ascontiguousarray(Wp[:, 2].reshape(48, CO))

    bm = b[:, :, 0, 0]                                    # [o,a]
    B2 = np.ascontiguousarray(bm.T.reshape(CO)).astype(np.float16)

    # BC[p, p'] = (p//30 == p'//30) for the squash row-sum broadcast matmul
    rows = np.arange(CP) // WC
    BC = (rows[:, None] == rows[None, :]).astype(np.float16)

    # im2col gather indices: rows (kh,kw,a); cols (d, h', w') stripped to 30x30
    a = np.arange(IA)
    d = np.arange(IC)
    hh = np.arange(HC)
    ww = np.arange(WC)

    def block_idx(khv, kwv):
        base = (a[None, :] * DCOL + khv[:, None] * WD + kwv[:, None]).reshape(-1)
        off = (d[:, None, None] * (IA * DCOL) + hh[None, :, None] * WD
               + ww[None, None, :]).reshape(-1)
        return base[:, None] + off[None, :]

    khA = np.repeat(np.arange(K), 2)
    kwA = np.tile(np.arange(2), K)
    idxA = block_idx(khA, kwA)          # [96, 7200]
    idxB = block_idx(np.arange(K), np.full(K, 2))  # [48, 7200]

    in_maps = []
    for k in range(NCORES):
        xs = x[k * NSAMP:(k + 1) * NSAMP].reshape(NSAMP, SAMP_ELEMS)
        XA = xs[:, idxA]                # [NSAMP, 96, 7200]
        XB = xs[:, idxB]                # [NSAMP, 48, 7200]
        in_maps.append({
            "XA": np.ascontiguousarray(XA),
            "XB": np.ascontiguousarray(XB),
            "WA": WA, "WB": WB, "B2": B2, "BC": np.ascontiguousarray(BC),
        })
    return in_maps


def _build_body(ctx, tc, aps):
    import concourse.bass as bass
    import concourse.mybir as mybir

    nc = tc.nc
    f32 = mybir.dt.float32
    f32r = mybir.dt.float32r
    f16 = mybir.dt.float16
    Alu = mybir.AluOpType
    Act = mybir.ActivationFunctionType
    X = mybir.AxisListType.X

    def pap(t, part, dims, off=0):
        return bass.AP(tensor=t.tensor, offset=t.offset + off,
                       ap=[[t.ap[0][0], part]] + dims)

    reps = int(os.environ.get("KREPS", "1"))

    consts = ctx.enter_context(tc.tile_pool(name="consts", bufs=1))
    ima_pool = ctx.enter_context(tc.tile_pool(name="ima", bufs=1))
    imb_pool = ctx.enter_context(tc.tile_pool(name="imb", bufs=1))
    vpool = ctx.enter_context(tc.tile_pool(name="votes", bufs=1))
    vvpool = ctx.enter_context(tc.tile_pool(name="vv", bufs=1))
    big1 = ctx.enter_context(tc.tile_pool(name="big1", bufs=1))    # t1/u1
    big2 = ctx.enter_context(tc.tile_pool(name="big2", bufs=1))    # t2/u2
    big3 = ctx.enter_context(tc.tile_pool(name="big3", bufs=1))    # s1/u3
    prep = ctx.enter_context(tc.tile_pool(name="prep", bufs=1))
    sm = ctx.enter_context(tc.tile_pool(name="sm", bufs=1))
    lpool = ctx.enter_context(tc.tile_pool(name="lp", bufs=2))
    actp = ctx.enter_context(tc.tile_pool(name="actp", bufs=1))
    psum_c = ctx.enter_context(tc.tile_pool(name="psc", bufs=1, space="PSUM"))
    psum_s = ctx.enter_context(tc.tile_pool(name="pss", bufs=1, space="PSUM"))

    # ---- constants ----
    wa = consts.tile([96, CO], f32r, tag="wa")
    nc.sync.dma_start(out=wa[:, :], in_=aps["WA"])
    wb = consts.tile([48, CO], f32r, tag="wb")
    nc.sync.dma_start(out=wb[:, :], in_=aps["WB"])
    bfull = consts.tile([128, CO], f16, tag="bfull")
    nc.sync.dma_start(out=bfull[:, :],
                      in_=bass.AP(tensor=aps["B2"].tensor, offset=0,
                                  ap=[[0, 128], [1, CO]]))
    bc = consts.tile([CP, CP], f16, tag="bc")
    nc.sync.dma_start(out=bc[:, :], in_=aps["BC"])
    zero_t = consts.tile([128, 1], f32, tag="zero")
    nc.vector.memset(zero_t[:, :], 0.0)
    eps_t = consts.tile([128, 1], f32, tag="eps")
    nc.vector.memset(eps_t[:, :], EPS)

    xa_ap, xb_ap = aps["XA"], aps["XB"]
    NF = NCH * VF                      # 16384: batched votes free size
    NP = NCH * CO                      # 2048: batched per-position free size
    NL = NCH * IC * OC                 # 1024: batched logits free size

    # engine split for the 8 per-chunk heavy muls (vr/va): DVE vs Pool
    DVE_CH = (0, 1, 2, 3, 4)
    for rep in range(reps):
      for s in range(NSAMP):
        # ---- input DMAs (host-prepped replicas) ----
        imA = ima_pool.tile([96, IC * HW], f32r, tag="imA")
        imB = imb_pool.tile([48, IC * HW], f32r, tag="imB")
        nc.sync.dma_start(out=imA[:, :], in_=bass.AP(
            tensor=xa_ap.tensor, offset=s * 96 * IC * HW,
            ap=[[IC * HW, 96], [1, IC * HW]]))
        nc.scalar.dma_start(out=imB[:, :], in_=bass.AP(
            tensor=xb_ap.tensor, offset=s * 48 * IC * HW,
            ap=[[IC * HW, 48], [1, IC * HW]]))

        # ---- conv: votes Vu[u] [120, (chunk4, i, a, o)] fp16 ----
        Vu = {}
        for u in range(2):
            Vu[u] = vpool.tile([CP, NCH // 2 * VF], f16, tag=f"Vb{u}",
                               name=f"Vb{u}")
            for j in range(NCH // 2):
                c = u * (NCH // 2) + j
                pc = psum_c.tile([CP, VF], f32, tag="pc")
                for d in range(IC):
                    off = d * HW + CSTART[c]
                    out_sl = pc[:, d * CO:(d + 1) * CO]
                    nc.tensor.matmul(out_sl, pap(imA, 96, [[1, CP]], off),
                                     wa[:, :], start=True, stop=False)
                    nc.tensor.matmul(out_sl, pap(imB, 48, [[1, CP]], off),
                                     wb[:, :], start=False, stop=True)
                nc.scalar.copy(out=Vu[u][:, j * VF:(j + 1) * VF], in_=pc[:, :])

        # ---- routing, iteration-major, batched per half-sample unit ----
        # 2 independent units of 4 chunks each -> deep cross-unit pipelining
        UCH = NCH // 2                 # 4 chunks per unit
        UF = UCH * VF                  # 8192
        UP = UCH * CO                  # 1024
        UL = UCH * IC * OC             # 512
        Lb = {}
        preb = {}
        scl = {}
        for it in range(ROUTINGS):
            for u in range(2):
                ch0 = u * UCH
                vboff = ch0 * VF
                # phase A -> preb[u] [120, (c4, a, o)] fp16
                preb[u] = prep.tile([CP, UP], f16, tag=f"preb{u}",
                                    name=f"preb{u}")
                if it == 0:
                    t1 = big1.tile([CP, UF // 2], f16, tag=f"t1u1{u}",
                                   name=f"t1a{u}")
                    nc.vector.tensor_tensor(
                        out=pap(t1, CP, [[4 * CO, UCH], [CO, 4], [1, CO]]),
                        in0=pap(Vu[u], CP, [[VF, UCH], [CO, 4], [1, CO]]),
                        in1=pap(Vu[u], CP, [[VF, UCH], [CO, 4], [1, CO]],
                                off=4 * CO),
                        op=Alu.add)
                    t2 = big2.tile([CP, UF // 4], f16, tag=f"t2u2{u}",
                                   name=f"t2a{u}")
                    nc.gpsimd.tensor_tensor(
                        out=pap(t2, CP, [[2 * CO, UCH], [CO, 2], [1, CO]]),
                        in0=pap(t1, CP, [[4 * CO, UCH], [CO, 2], [1, CO]]),
                        in1=pap(t1, CP, [[4 * CO, UCH], [CO, 2], [1, CO]],
                                off=2 * CO),
                        op=Alu.add)
                    s1 = big3.tile([CP, UF // 8], f16, tag=f"s1u3{u}",
                                   name=f"s1a{u}")
                    nc.vector.tensor_tensor(
                        out=pap(s1, CP, [[CO, UCH], [1, CO]]),
                        in0=pap(t2, CP, [[2 * CO, UCH], [1, CO]]),
                        in1=pap(t2, CP, [[2 * CO, UCH], [1, CO]], off=CO),
                        op=Alu.add)
                    nc.vector.scalar_tensor_tensor(
                        out=preb[u][:, :], in0=s1[:, :], scalar=1.0 / OC,
                        in1=pap(bfull, CP, [[0, UCH], [1, CO]]),
                        op0=Alu.mult, op1=Alu.add)
                else:
                    eb = sm.tile([CP, UL], f32, tag=f"eb{u}", name=f"eb{u}")
                    nc.scalar.activation(out=eb[:, :], in_=Lb[u][:, :],
                                         func=Act.Exp, bias=zero_t[:CP, :])
                    ssum = sm.tile([CP, UCH * IC], f32, tag=f"ssum{u}",
                                   name=f"ssum{u}")
                    nc.vector.tensor_reduce(
                        out=ssum[:, :],
                        in_=pap(eb, CP, [[IC * OC, UCH], [OC, IC], [1, OC]]),
                        axis=X, op=Alu.add)
                    rs = sm.tile([CP, UCH * IC], f32, tag=f"rs{u}",
                                 name=f"rs{u}")
                    nc.vector.reciprocal(out=rs[:, :], in_=ssum[:, :])
                    rb = sm.tile([CP, UL], f16, tag=f"rb{u}", name=f"rb{u}")
                    nc.gpsimd.tensor_tensor(
                        out=pap(rb, CP, [[IC * OC, UCH], [OC, IC], [1, OC]]),
                        in0=pap(eb, CP, [[IC * OC, UCH], [OC, IC], [1, OC]]),
                        in1=pap(rs, CP, [[IC, UCH], [1, IC], [0, OC]]),
                        op=Alu.mult)
                    vvb = vvpool.tile([CP, UF], f16, tag=f"vv{u}",
                                      name=f"vv{u}")
                    for j in range(UCH):
                        eng = nc.vector if j % 2 == 0 else nc.gpsimd
                        eng.tensor_tensor(
                            out=pap(vvb, CP, [[CO, IC], [NA, OC], [1, NA]],
                                    off=j * VF),
                            in0=pap(Vu[u], CP, [[CO, IC], [NA, OC], [1, NA]],
                                    off=j * VF),
                            in1=pap(rb, CP, [[OC, IC], [0, OC], [1, OC]],
                                    off=j * IC * OC),
                            op=Alu.mult)
                    t1 = big1.tile([CP, UF // 2], f16, tag=f"t1u1{u}",
                                   name=f"t1b{u}")
                    nc.vector.tensor_tensor(
                        out=pap(t1, CP, [[4 * CO, UCH], [CO, 4], [1, CO]]),
                        in0=pap(vvb, CP, [[VF, UCH], [CO, 4], [1, CO]]),
                        in1=pap(vvb, CP, [[VF, UCH], [CO, 4], [1, CO]],
                                off=4 * CO),
                        op=Alu.add)
                    t2 = big2.tile([CP, UF // 4], f16, tag=f"t2u2{u}",
                                   name=f"t2b{u}")
                    nc.gpsimd.tensor_tensor(
                        out=pap(t2, CP, [[2 * CO, UCH], [CO, 2], [1, CO]]),
                        in0=pap(t1, CP, [[4 * CO, UCH], [CO, 2], [1, CO]]),
                        in1=pap(t1, CP, [[4 * CO, UCH], [CO, 2], [1, CO]],
                                off=2 * CO),
                        op=Alu.add)
                    s1 = big3.tile([CP, UF // 8], f16, tag=f"s1u3{u}",
                                   name=f"s1b{u}")
                    nc.vector.tensor_tensor(
                        out=pap(s1, CP, [[CO, UCH], [1, CO]]),
                        in0=pap(t2, CP, [[2 * CO, UCH], [1, CO]]),
                        in1=pap(t2, CP, [[2 * CO, UCH], [1, CO]], off=CO),
                        op=Alu.add)
                    nc.gpsimd.tensor_tensor(
                        out=preb[u][:, :], in0=s1[:, :],
                        in1=pap(bfull, CP, [[0, UCH], [1, CO]]), op=Alu.add)

            for u in range(2):
                ch0 = u * UCH
                vboff = ch0 * VF
                # phase B: squash (+ agreement or output)
                sqb = sm.tile([CP, UP], f16, tag=f"sqb{u}", name=f"sqb{u}")
                nc.scalar.activation(out=sqb[:, :], in_=preb[u][:, :],
                                     func=Act.Square, bias=zero_t[:CP, :])
                s2p = psum_s.tile([CP, UP], f32, tag=f"s2p{u}",
                                  name=f"s2p{u}")
                for j in range(UCH):
                    nc.tensor.matmul(s2p[:, j * CO:(j + 1) * CO], bc[:, :],
                                     pap(sqb, CP, [[1, CO]], off=j * CO),
                                     start=True, stop=True)
                s2s = sm.tile([CP, UP], f32, tag=f"s2s{u}", name=f"s2s{u}")
                nc.scalar.copy(out=s2s[:, :], in_=s2p[:, :])
                sq1 = sm.tile([CP, UP], f32, tag=f"sq1{u}", name=f"sq1{u}")
                nc.scalar.activation(out=sq1[:, :], in_=s2s[:, :],
                                     func=Act.Sqrt, bias=eps_t[:CP, :])
                den = sm.tile([CP, UP], f32, tag=f"den{u}", name=f"den{u}")
                nc.vector.scalar_tensor_tensor(
                    out=den[:, :], in0=s2s[:, :], scalar=1.0, in1=sq1[:, :],
                    op0=Alu.add, op1=Alu.mult)
                rden = sm.tile([CP, UP], f16, tag=f"rden{u}", name=f"rden{u}")
                nc.vector.reciprocal(out=rden[:, :], in_=den[:, :])
                scl[u] = sm.tile([CP, UP], f16, tag=f"scl{u}", name=f"scl{u}")
                nc.gpsimd.tensor_tensor(out=scl[u][:, :], in0=s2s[:, :],
                                        in1=rden[:, :], op=Alu.mult)
                if it < ROUTINGS - 1:
                    actb = sm.tile([CP, UP], f16, tag=f"actb{u}",
                                   name=f"actb{u}")
                    nc.gpsimd.tensor_tensor(out=actb[:, :], in0=preb[u][:, :],
                                            in1=scl[u][:, :], op=Alu.mult)
                    vvb = vvpool.tile([CP, UF], f16, tag=f"vv{u}",
                                      name=f"vvb{u}")
                    for j in range(UCH):
                        eng = nc.vector if j % 2 == 1 else nc.gpsimd
                        eng.tensor_tensor(
                            out=pap(vvb, CP, [[CO, IC], [1, CO]], off=j * VF),
                            in0=pap(Vu[u], CP, [[CO, IC], [1, CO]],
                                    off=j * VF),
                            in1=pap(actb, CP, [[0, IC], [1, CO]], off=j * CO),
                            op=Alu.mult)
                    u1 = big1.tile([CP, UF // 2], f16, tag=f"t1u1{u}",
                                   name=f"u1{u}")
                    nc.vector.tensor_tensor(
                        out=pap(u1, CP, [[IC * 128, UCH], [128, IC], [1, 128]]),
                        in0=pap(vvb, CP, [[VF, UCH], [CO, IC], [1, 128]]),
                        in1=pap(vvb, CP, [[VF, UCH], [CO, IC], [1, 128]],
                                off=128),
                        op=Alu.add)
                    u2 = big2.tile([CP, UF // 4], f16, tag=f"t2u2{u}",
                                   name=f"u2{u}")
                    nc.vector.tensor_tensor(
                        out=pap(u2, CP, [[IC * 64, UCH], [64, IC], [1, 64]]),
                        in0=pap(u1, CP, [[IC * 128, UCH], [128, IC], [1, 64]]),
                        in1=pap(u1, CP, [[IC * 128, UCH], [128, IC], [1, 64]],
                                off=64),
                        op=Alu.add)
                    u3 = big3.tile([CP, UF // 8], f16, tag=f"s1u3{u}",
                                   name=f"u3{u}")
                    nc.gpsimd.tensor_tensor(
                        out=pap(u3, CP, [[IC * 32, UCH], [32, IC], [1, 32]]),
                        in0=pap(u2, CP, [[IC * 64, UCH], [64, IC], [1, 32]]),
                        in1=pap(u2, CP, [[IC * 64, UCH], [64, IC], [1, 32]],
                                off=32),
                        op=Alu.add)
                    lnew = lpool.tile([CP, UL], f16, tag=f"Lb{u}",
                                      name=f"Lb{u}")
                    if it == 0:
                        nc.vector.tensor_tensor(
                            out=pap(lnew, CP,
                                    [[IC * OC, UCH], [OC, IC], [1, OC]]),
                            in0=pap(u3, CP, [[IC * 32, UCH], [32, IC], [1, OC]]),
                            in1=pap(u3, CP, [[IC * 32, UCH], [32, IC], [1, OC]],
                                    off=OC),
                            op=Alu.add)
                    else:
                        nc.vector.tensor_tensor(
                            out=pap(lnew, CP,
                                    [[IC * OC, UCH], [OC, IC], [1, OC]]),
                            in0=pap(u3, CP, [[IC * 32, UCH], [32, IC], [1, OC]]),
                            in1=pap(u3, CP, [[IC * 32, UCH], [32, IC], [1, OC]],
                                    off=OC),
                            op=Alu.add)
                        nc.gpsimd.tensor_tensor(out=lnew[:, :],
                                                in0=lnew[:, :],
                                                in1=Lb[u][:, :], op=Alu.add)
                    Lb[u] = lnew
                else:
                    for j in range(UCH):
                        c = ch0 + j
                        actf = actp.tile([CP, CO], f32, tag="actf",
                                         name="actf")
                        nc.gpsimd.tensor_tensor(
                            out=pap(actf, CP, [[1, NA], [NA, OC]]),
                            in0=pap(preb[u], CP, [[OC, NA], [1, OC]],
                                    off=j * CO),
                            in1=pap(scl[u], CP, [[OC, NA], [1, OC]],
                                    off=j * CO),
                            op=Alu.mult)
                        if c < NCH - 1:
                            dst = bass.AP(tensor=aps["out"].tensor,
                                          offset=s * (HW * CO) + CSTART[c] * CO,
                                          ap=[[CO, CP], [1, CO]])
                            nc.sync.dma_start(out=dst, in_=actf[:, :])
                        else:
                            dst = bass.AP(tensor=aps["out"].tensor,
                                          offset=s * (HW * CO) + (HW - 60) * CO,
                                          ap=[[CO, 60], [1, CO]])
                            nc.sync.dma_start(out=dst, in_=actf[60:, :])


_CACHED = None


def _build():
    global _CACHED
    if _CACHED is not None:
        return _CACHED
    from contextlib import ExitStack
    import concourse.bacc as bacc
    import concourse.mybir as mybir
    import concourse.tile as tile

    nc = bacc.Bacc("TRN2", target_bir_lowering=False, debug=False,
                   num_devices=NCORES)
    f32 = mybir.dt.float32
    f32r = mybir.dt.float32r
    f16 = mybir.dt.float16
    xa_t = nc.dram_tensor("XA", [NSAMP, 96, IC * HW], f32r, kind="ExternalInput")
    xb_t = nc.dram_tensor("XB", [NSAMP, 48, IC * HW], f32r, kind="ExternalInput")
    wa_t = nc.dram_tensor("WA", [96, CO], f32r, kind="ExternalInput")
    wb_t = nc.dram_tensor("WB", [48, CO], f32r, kind="ExternalInput")
    b2_t = nc.dram_tensor("B2", [CO], f16, kind="ExternalInput")
    bc_t = nc.dram_tensor("BC", [CP, CP], f16, kind="ExternalInput")
    out_t = nc.dram_tensor("out", [NSAMP, HC, WC, OC, NA], f32, kind="ExternalOutput")

    aps = {"XA": xa_t.ap(), "XB": xb_t.ap(), "WA": wa_t.ap(), "WB": wb_t.ap(),
           "B2": b2_t.ap(), "BC": bc_t.ap(), "out": out_t.ap()}
    with nc.allow_low_precision(reason="fp16 routing validated vs reference"):
        with tile.TileContext(nc) as tc:
            with ExitStack() as ctx:
                _build_body(ctx, tc, aps)
    nc.compile()
    _CACHED = nc
    return nc


def run(x, W, b, trace=False):
    from concourse.bass_utils import run_bass_kernel_spmd

    nc = _build()
    in_maps = make_in_maps(x, W, b)
    res = run_bass_kernel_spmd(nc, in_maps, core_ids=list(range(NCORES)),
                               trace=trace)
    out = np.concatenate([res.results[k]["out"] for k in range(NCORES)], axis=0)
    return out.reshape(B, HC, WC, OC, NA), res


def kernel(x, W, b):
    out, _ = run(x, W, b, trace=False)
    return out.astype(np.float32)


if __name__ == "__main__":
    nc = _build()
    print("built ok")


# revision 25
# speedup vs baseline: 1.9919x; 1.1761x over previous
"""Trainium2 Bass kernel for nn_ConvCapsuleLayer3D.

Self-contained: takes FULL inputs x[32,32,32,8,16], W[16,3,3,1,256], b[16,16,1,1],
returns FULL output [32,30,30,16,16] (fp32). Data-parallel over batch across 8
NeuronCores (4 samples each).

Host prep per core: stripped im2col replicas XA[(kh,kw<2,a)=96, (d,h',w')=7200]
and XB[(kh,kw=2,a)=48, 7200] so conv chunk windows are contiguous stationary
APs; weights pre-permuted to (a_out-major, o-minor) columns; fp16 constants.

Device per sample: conv = 2 accumulating f32r matmuls per (d, chunk) -> votes
V[hw_chunk, (i,a,o)] drained to fp16. Dynamic routing (3 iters, iteration-major
with iter-0 uniform-softmax shortcut) per 120-position chunk in fp16: DVE 2x
tensor_tensor for heavy muls + tree-adds for the i/a reductions, squash row-sums
via a 0/1 broadcast matmul on TensorE, remaining elementwise on GPSIMD/ACT.
"""
import os
import sys

import numpy as np

sys.path.insert(0, "/opt/trn_rl_repo")

# --- problem constants (hardcoded; kernel.py must not read /root/problem) ---
B, H, WD, IC, IA = 32, 32, 32, 8, 16
OC, NA = 16, 16
K = 3
HC, WC = H - K + 1, WD - K + 1       # 30, 30
HW = HC * WC                         # 900
CO = OC * NA                         # 256
NCORES = 8
NSAMP = B // NCORES                  # 4
EPS = 1e-7
ROUTINGS = 3

CP_FULL = 120                        # 4 h-rows per chunk
CHUNKS = [(c, CP_FULL, 4) for c in range(7)] + [(7, 60, 2)]  # (c, cp, nj)
NCH = len(CHUNKS)
SAMP_ELEMS = H * WD * IC * IA        # 131072
DCOL = 1024                          # per-(d,a) column run in x

INPUT_NAMES = ["XA", "XB", "WA", "WB", "B2", "BC"]


def make_in_maps(x, W, b):
    """Host prep: per-core input dicts for run_bass_kernel_spmd."""
    x = np.ascontiguousarray(x, np.float32)
    W = np.ascontiguousarray(W, np.float32)
    b = np.ascontiguousarray(b, np.float32)

    W5 = W[:, :, :, 0, :].reshape(IA, K, K, OC, NA)       # [a,kh,kw,o,ao]
    Wp = W5.transpose(1, 2, 0, 4, 3)                      # [kh,kw,a,ao,o]
    WA = np.ascontiguousarray(Wp[:, :2].reshape(96, CO))
    WB = np.ascontiguousarray(Wp[:, 2].reshape(48, CO))

    bm = b[:, :, 0, 0]                                    # [o,a]
    B2 = np.ascontiguousarray(bm.T.reshape(CO)).astype(np.float16)

    rows = np.arange(CP_FULL) // WC
    BC = (rows[:, None] == rows[None, :]).astype(np.float16)

    a = np.arange(IA)
    d = np.arange(IC)
    hh = np.arange(HC)
    ww = np.arange(WC)

    def block_idx(khv, kwv):
        base = (a[None, :] * DCOL + khv[:, None] * WD + kwv[:, None]).reshape(-1)
        off = (d[:, None, None] * (IA * DCOL) + hh[None, :, None] * WD
               + ww[None, None, :]).reshape(-1)
        return base[:, None] + off[None, :]

    khA = np.repeat(np.arange(K), 2)
    kwA = np.tile(np.arange(2), K)
    idxA = block_idx(khA, kwA)          # [96, 7200]
    idxB = block_idx(np.arange(K), np.full(K, 2))  # [48, 7200]

    in_maps = []
    for k in range(NCORES):
        xs = x[k * NSAMP:(k + 1) * NSAMP].reshape(NSAMP, SAMP_ELEMS)
        XA = xs[:, idxA]
        XB = xs[:, idxB]
        in_maps.append({
            "XA": np.ascontiguousarray(XA),
            "XB": np.ascontiguousarray(XB),
            "WA": WA, "WB": WB, "B2": B2, "BC": np.ascontiguousarray(BC),
        })
    return in_maps


def _build_body(ctx, tc, aps):
    import concourse.bass as bass
    import concourse.mybir as mybir

    nc = tc.nc
    f32 = mybir.dt.float32
    f32r = mybir.dt.float32r
    f16 = mybir.dt.float16
    Alu = mybir.AluOpType
    Act = mybir.ActivationFunctionType
    X = mybir.AxisListType.X

    def pap(t, part, dims, off=0):
        return bass.AP(tensor=t.tensor, offset=t.offset + off,
                       ap=[[t.ap[0][0], part]] + dims)

    reps = int(os.environ.get("KREPS", "1"))

    consts = ctx.enter_context(tc.tile_pool(name="consts", bufs=1))
    ima_pool = ctx.enter_context(tc.tile_pool(name="ima", bufs=1))
    imb_pool = ctx.enter_context(tc.tile_pool(name="imb", bufs=1))
    vpool = ctx.enter_context(tc.tile_pool(name="votes", bufs=1))
    big = ctx.enter_context(tc.tile_pool(name="big", bufs=3))      # vr/va/t1/u1
    mid = ctx.enter_context(tc.tile_pool(name="mid", bufs=3))      # t2/u2/u3/s1
    sm = ctx.enter_context(tc.tile_pool(name="sm", bufs=3))        # e/r/sq/...
    prep = ctx.enter_context(tc.tile_pool(name="prep", bufs=2))
    lpool = ctx.enter_context(tc.tile_pool(name="lp", bufs=2))
    actp = ctx.enter_context(tc.tile_pool(name="actp", bufs=3))
    psum_c = ctx.enter_context(tc.tile_pool(name="psc", bufs=1, space="PSUM"))
    psum_s = ctx.enter_context(tc.tile_pool(name="pss", bufs=2, space="PSUM"))

    # ---- constants ----
    wa = consts.tile([96, CO], f32r, tag="wa")
    nc.sync.dma_start(out=wa[:, :], in_=aps["WA"])
    wb = consts.tile([48, CO], f32r, tag="wb")
    nc.sync.dma_start(out=wb[:, :], in_=aps["WB"])
    bfull = consts.tile([128, CO], f16, tag="bfull")
    nc.sync.dma_start(out=bfull[:, :],
                      in_=bass.AP(tensor=aps["B2"].tensor, offset=0,
                                  ap=[[0, 128], [1, CO]]))
    bc = consts.tile([CP_FULL, CP_FULL], f16, tag="bc")
    nc.sync.dma_start(out=bc[:, :], in_=aps["BC"])
    zero_t = consts.tile([128, 1], f32, tag="zero")
    nc.vector.memset(zero_t[:, :], 0.0)
    eps_t = consts.tile([128, 1], f32, tag="eps")
    nc.vector.memset(eps_t[:, :], EPS)

    xa_ap, xb_ap = aps["XA"], aps["XB"]

    for rep in range(reps):
      for s in range(NSAMP):
        imA = ima_pool.tile([96, IC * HW], f32r, tag="imA")
        imB = imb_pool.tile([48, IC * HW], f32r, tag="imB")
        nc.sync.dma_start(out=imA[:, :], in_=bass.AP(
            tensor=xa_ap.tensor, offset=s * 96 * IC * HW,
            ap=[[IC * HW, 96], [1, IC * HW]]))
        nc.scalar.dma_start(out=imB[:, :], in_=bass.AP(
            tensor=xb_ap.tensor, offset=s * 48 * IC * HW,
            ap=[[IC * HW, 48], [1, IC * HW]]))

        # ---- conv: votes V[c] [cp, (i,a,o)] fp16 ----
        V = {}
        for (c, cp, nj) in CHUNKS:
            pc = psum_c.tile([CP_FULL, IC * CO], f32, tag="pc")
            for d in range(IC):
                off = d * HW + c * CP_FULL
                out_sl = pc[:cp, d * CO:(d + 1) * CO]
                nc.tensor.matmul(out_sl, pap(imA, 96, [[1, cp]], off),
                                 wa[:, :], start=True, stop=False)
                nc.tensor.matmul(out_sl, pap(imB, 48, [[1, cp]], off),
                                 wb[:, :], start=False, stop=True)
            V[c] = vpool.tile([CP_FULL, IC * CO], f16, tag=f"V{c}", name=f"V{c}")
            nc.scalar.copy(out=V[c][:cp, :], in_=pc[:cp, :])

        # ---- routing, iteration-major ----
        L = {}
        pre = {}
        for it in range(ROUTINGS):
            for (c, cp, nj) in CHUNKS:
                if it == 0:
                    t1 = big.tile([CP_FULL, 4 * CO], f16, tag="t1")
                    nc.vector.tensor_tensor(
                        out=pap(t1, cp, [[CO, 4], [1, CO]]),
                        in0=pap(V[c], cp, [[CO, 4], [1, CO]]),
                        in1=pap(V[c], cp, [[CO, 4], [1, CO]], off=4 * CO),
                        op=Alu.add)
                    t2 = mid.tile([CP_FULL, 2 * CO], f16, tag="t2")
                    nc.gpsimd.tensor_tensor(
                        out=pap(t2, cp, [[CO, 2], [1, CO]]),
                        in0=pap(t1, cp, [[CO, 2], [1, CO]]),
                        in1=pap(t1, cp, [[CO, 2], [1, CO]], off=2 * CO),
                        op=Alu.add)
                    s1 = mid.tile([CP_FULL, CO], f16, tag="s1")
                    nc.vector.tensor_tensor(
                        out=s1[:cp, :], in0=t2[:cp, 0:CO], in1=t2[:cp, CO:2 * CO],
                        op=Alu.add)
                    pre[c] = prep.tile([CP_FULL, CO], f16, tag=f"pre{c}",
                                       name=f"pre{c}")
                    nc.vector.scalar_tensor_tensor(
                        out=pre[c][:cp, :], in0=s1[:cp, :], scalar=1.0 / OC,
                        in1=pap(bfull, cp, [[1, CO]]), op0=Alu.mult, op1=Alu.add)
                else:
                    e = sm.tile([CP_FULL, IC * OC], f32, tag="e")
                    nc.scalar.activation(out=e[:cp, :], in_=L[c][:cp, :],
                                         func=Act.Exp, bias=zero_t[:cp, :])
                    ssum = sm.tile([CP_FULL, IC], f32, tag="ssum")
                    nc.vector.tensor_reduce(
                        out=ssum[:cp, :],
                        in_=pap(e, cp, [[OC, IC], [1, OC]]), axis=X, op=Alu.add)
                    rs = sm.tile([CP_FULL, IC], f32, tag="rs")
                    nc.vector.reciprocal(out=rs[:cp, :], in_=ssum[:cp, :])
                    r = sm.tile([CP_FULL, IC * OC], f16, tag="r")
                    nc.gpsimd.tensor_tensor(
                        out=pap(r, cp, [[OC, IC], [1, OC]]),
                        in0=pap(e, cp, [[OC, IC], [1, OC]]),
                        in1=pap(rs, cp, [[1, IC], [0, OC]]),
                        op=Alu.mult)
                    vr = big.tile([CP_FULL, IC * CO], f16, tag="vr")
                    nc.vector.tensor_tensor(
                        out=pap(vr, cp, [[CO, IC], [NA, OC], [1, NA]]),
                        in0=pap(V[c], cp, [[CO, IC], [NA, OC], [1, NA]]),
                        in1=pap(r, cp, [[OC, IC], [0, OC], [1, OC]]),
                        op=Alu.mult)
                    t1 = big.tile([CP_FULL, 4 * CO], f16, tag="t1")
                    nc.vector.tensor_tensor(
                        out=pap(t1, cp, [[CO, 4], [1, CO]]),
                        in0=pap(vr, cp, [[CO, 4], [1, CO]]),
                        in1=pap(vr, cp, [[CO, 4], [1, CO]], off=4 * CO),
                        op=Alu.add)
                    t2 = mid.tile([CP_FULL, 2 * CO], f16, tag="t2")
                    nc.gpsimd.tensor_tensor(
                        out=pap(t2, cp, [[CO, 2], [1, CO]]),
                        in0=pap(t1, cp, [[CO, 2], [1, CO]]),
                        in1=pap(t1, cp, [[CO, 2], [1, CO]], off=2 * CO),
                        op=Alu.add)
                    s1 = mid.tile([CP_FULL, CO], f16, tag="s1")
                    nc.vector.tensor_tensor(
                        out=s1[:cp, :], in0=t2[:cp, 0:CO], in1=t2[:cp, CO:2 * CO],
                        op=Alu.add)
                    pre[c] = prep.tile([CP_FULL, CO], f16, tag=f"pre{c}",
                                       name=f"pre{c}")
                    nc.gpsimd.tensor_tensor(
                        out=pre[c][:cp, :], in0=s1[:cp, :],
                        in1=pap(bfull, cp, [[1, CO]]), op=Alu.add)

            for (c, cp, nj) in CHUNKS:
                sq = sm.tile([CP_FULL, CO], f16, tag="sq")
                nc.scalar.activation(out=sq[:cp, :], in_=pre[c][:cp, :],
                                     func=Act.Square, bias=zero_t[:cp, :])
                s2p = psum_s.tile([CP_FULL, CO], f32, tag="s2p")
                nc.tensor.matmul(s2p[:cp, :], bc[:cp, :cp], sq[:cp, :],
                                 start=True, stop=True)
                s2s = sm.tile([CP_FULL, CO], f32, tag="s2s")
                nc.scalar.copy(out=s2s[:cp, :], in_=s2p[:cp, :])
                sq1 = sm.tile([CP_FULL, CO], f32, tag="sq1")
                nc.scalar.activation(out=sq1[:cp, :], in_=s2s[:cp, :],
                                     func=Act.Sqrt, bias=eps_t[:cp, :])
                den = sm.tile([CP_FULL, CO], f32, tag="den")
                nc.vector.scalar_tensor_tensor(
                    out=den[:cp, :], in0=s2s[:cp, :], scalar=1.0,
                    in1=sq1[:cp, :], op0=Alu.add, op1=Alu.mult)
                rden = sm.tile([CP_FULL, CO], f16, tag="rden")
                nc.vector.reciprocal(out=rden[:cp, :], in_=den[:cp, :])
                scl = sm.tile([CP_FULL, CO], f16, tag="scl")
                nc.gpsimd.tensor_tensor(out=scl[:cp, :], in0=s2s[:cp, :],
                                        in1=rden[:cp, :], op=Alu.mult)
                if it < ROUTINGS - 1:
                    act = sm.tile([CP_FULL, CO], f16, tag="act")
                    nc.gpsimd.tensor_tensor(out=act[:cp, :], in0=pre[c][:cp, :],
                                            in1=scl[:cp, :], op=Alu.mult)
                    va = big.tile([CP_FULL, IC * CO], f16, tag="va")
                    nc.vector.tensor_tensor(
                        out=pap(va, cp, [[CO, IC], [1, CO]]),
                        in0=pap(V[c], cp, [[CO, IC], [1, CO]]),
                        in1=pap(act, cp, [[0, IC], [1, CO]]),
                        op=Alu.mult)
                    u1 = big.tile([CP_FULL, IC * 128], f16, tag="u1")
                    nc.vector.tensor_tensor(
                        out=pap(u1, cp, [[128, IC], [1, 128]]),
                        in0=pap(va, cp, [[CO, IC], [1, 128]]),
                        in1=pap(va, cp, [[CO, IC], [1, 128]], off=128),
                        op=Alu.add)
                    u2 = mid.tile([CP_FULL, IC * 64], f16, tag="u2")
                    nc.gpsimd.tensor_tensor(
                        out=pap(u2, cp, [[64, IC], [1, 64]]),
                        in0=pap(u1, cp, [[128, IC], [1, 64]]),
                        in1=pap(u1, cp, [[128, IC], [1, 64]], off=64),
                        op=Alu.add)
                    u3 = mid.tile([CP_FULL, IC * 32], f16, tag="u3")
                    nc.gpsimd.tensor_tensor(
                        out=pap(u3, cp, [[32, IC], [1, 32]]),
                        in0=pap(u2, cp, [[64, IC], [1, 32]]),
                        in1=pap(u2, cp, [[64, IC], [1, 32]], off=32),
                        op=Alu.add)
                    lnew = lpool.tile([CP_FULL, IC * OC], f16, tag=f"L{c}",
                                      name=f"L{c}")
                    nc.vector.tensor_tensor(
                        out=pap(lnew, cp, [[OC, IC], [1, OC]]),
                        in0=pap(u3, cp, [[32, IC], [1, OC]]),
                        in1=pap(u3, cp, [[32, IC], [1, OC]], off=OC),
                        op=Alu.add)
                    if it > 0:
                        nc.gpsimd.tensor_tensor(out=lnew[:cp, :],
                                                in0=lnew[:cp, :],
                                                in1=L[c][:cp, :], op=Alu.add)
                    L[c] = lnew
                else:
                    actf = actp.tile([CP_FULL, CO], f32, tag="actf",
                                     name="actf")
                    nc.gpsimd.tensor_tensor(
                        out=pap(actf, cp, [[1, NA], [NA, OC]]),
                        in0=pap(pre[c], cp, [[OC, NA], [1, OC]]),
                        in1=pap(scl, cp, [[OC, NA], [1, OC]]),
                        op=Alu.mult)
                    dst = bass.AP(tensor=aps["out"].tensor,
                                  offset=s * (HW * CO) + c * (CP_FULL * CO),
                                  ap=[[CO, cp], [1, CO]])
                    nc.sync.dma_start(out=dst, in_=actf[:cp, :])


_CACHED = None


def _build():
    global _CACHED
    if _CACHED is not None:
        return _CACHED
    from contextlib import ExitStack
    import concourse.bacc as bacc
    import concourse.mybir as mybir
    import concourse.tile as tile

    nc = bacc.Bacc("TRN2", target_bir_lowering=False, debug=False,
                   num_devices=NCORES)
    f32 = mybir.dt.float32
    f32r = mybir.dt.float32r
    f16 = mybir.dt.float16
    xa_t = nc.dram_tensor("XA", [NSAMP, 96, IC * HW], f32r, kind="ExternalInput")
    xb_t = nc.dram_tensor("XB", [NSAMP, 48, IC * HW], f32r, kind="ExternalInput")
    wa_t = nc.dram_tensor("WA", [96, CO], f32r, kind="ExternalInput")
    wb_t = nc.dram_tensor("WB", [48, CO], f32r, kind="ExternalInput")
    b2_t = nc.dram_tensor("B2", [CO], f16, kind="ExternalInput")
    bc_t = nc.dram_tensor("BC", [CP_FULL, CP_FULL], f16, kind="ExternalInput")
    out_t = nc.dram_tensor("out", [NSAMP, HC, WC, OC, NA], f32, kind="ExternalOutput")

    aps = {"XA": xa_t.ap(), "XB": xb_t.ap(), "WA": wa_t.ap(), "WB": wb_t.ap(),
           "B2": b2_t.ap(), "BC": bc_t.ap(), "out": out_t.ap()}
    with nc.allow_low_precision(reason="fp16 routing validated vs reference"):
        with tile.TileContext(nc) as tc:
            with ExitStack() as ctx:
                _build_body(ctx, tc, aps)
    nc.compile()
    _CACHED = nc
    return nc


def run(x, W, b, trace=False):
    from concourse.bass_utils import run_bass_kernel_spmd

    nc = _build()
    in_maps = make_in_maps(x, W, b)
    res = run_bass_kernel_spmd(nc, in_maps, core_ids=list(range(NCORES)),
                               trace=trace)
    out = np.concatenate([res.results[k]["out"] for k in range(NCORES)], axis=0)
    return out.reshape(B, HC, WC, OC, NA), res


def kernel(x, W, b):
    out, _ = run(x, W, b, trace=False)
    return out.astype(np.float32)


if __name__ == "__main__":
    nc = _build()
    print("built ok")
